# revision 7
# baseline (speedup 1.0000x reference)
"""Distillation-trainer loss kernel for Trainium2 (8 NeuronCores), fp8 edition.

Computes  loss = mean((attn(q,k,v) - attn(q,ck,cv))**2)  for
q:[1,8,1024,128], k/v:[1,8,8192,128], ck/cv:[1,8,1024,128] fp32.

Sharding: one kv-head per core (h axis, 8 heads / 8 cores). Each core
computes its head's squared-error partial sums; the host adds the 8
partials and divides by the element count.

Speed model: the PE runs fp8 matmuls in MatmulPerfMode.DoubleRow at
0.5 cycles/output-row (2-4x bf16):
  - QK: contraction d=128 split as two 64-planes -> stationary
    kT8[64,2,128n] x moving q8[64,2,256q] -> scoresT [128n, 256q].
  - PV: two n-tiles per instruction (K=256) -> stationary
    probs[128n,2,128q] x moving v8[128n,2,129] -> accum z'|S [128q,129].
q/k/v in e4m3. Softmax probs in e5m2 (its wider exponent range covers
the score span [-6.9, 8.6]); all probs carry a uniform e^-SHIFT factor
that cancels in the softmax ratio and keeps exp outputs inside fp8
range. Mixed e5m2-stationary x e4m3-moving DoubleRow is HW-validated.

Exp is the new bottleneck (~9.4M elements; only ACT and DVE can read
PSUM): teacher groups alternate
  ACT:  probs = Exp(s*SCALE - SHIFT) -> e5m2 (even groups + compressed)
  DVE:  Schraudolph: i8 = sat(rint(s*A5 + B5)) bitcast e5m2 (odd groups)
GPSIMD (no PSUM access) takes the SBUF-side normalize/MSE math so DVE
stays on exp; tiny reciprocals stay on DVE.

Per-core algorithm per 256-wide q-chunk: compressed attend then teacher
attend, PV lagging QK by 2 groups; ACT drains z'|S PSUM->SBUF inside
the next attend's group loop; GPS computes zcomp = z'c/Sc and
acc += sum((z'*invS - zcomp)^2) deferred into the following chunk.
"""

import numpy as np
import ml_dtypes

import concourse.bass as bass
import concourse.mybir as mybir
import concourse.tile as tile
from concourse import bacc
from concourse.bass_utils import run_bass_kernel_spmd

F32 = mybir.dt.float32
I8 = mybir.dt.int8
E4 = mybir.dt.float8e4
E5 = mybir.dt.float8e5
AF = mybir.ActivationFunctionType
ALU = mybir.AluOpType
DR = mybir.MatmulPerfMode.DoubleRow

B, H, Q, N, NC, D = 1, 8, 1024, 8192, 1024, 128
N_CORES = 8
SCALE = 1.0 / float(np.sqrt(D))

QC = 256                   # q chunk width
N_QC = Q // QC             # 4
GT = 4                     # n-tiles per PSUM scores group (2 banks)
NT = N // 128              # 64 teacher n-tiles
NCT = NC // 128            # 8 compressed n-tiles
PV_LAG = 2                 # groups of PV emission lag behind QK

# Softmax range management: probs carry e^-SHIFT (cancels in z = P@V/S).
# Score*SCALE spans [-6.95, 8.63] for this input distribution; with
# SHIFT=2.8 the ACT path tops out at exp(5.83)=340 (e5m2 max 57344) and
# the DVE Schraudolph bits stay in [3, 94] - no saturation anywhere.
LN2 = float(np.log(2.0))
SHIFT = 2.8
A5 = 4.0 * SCALE / LN2                       # bits per raw-score unit
B5 = 4.0 * (15.0 - SHIFT / LN2) - 0.25       # badj=-0.25 (tuned in sim)


def teacher_on_dve(qc, gi):
    # ~7.5/16 of teacher groups on DVE: DVE also runs the MSE/normalize
    # math, ACT also runs the compressed groups + drains.
    return gi % 2 == 1 and not (qc % 2 == 1 and gi == 15)


def comp_on_dve(qc, gi):
    return False


def _emit(nc: bass.Bass, tc: tile.TileContext, qTh, kTh, vbh, ckTh, cvbh, out_dram):
    ctxs = []

    def pool(**kw):
        p = tc.tile_pool(**kw)
        ctxs.append(p)
        return p.__enter__()

    pconst = pool(name="pconst", bufs=1)
    pex = pool(name="pex", bufs=4)
    psmall = pool(name="psmall", bufs=8)
    psc = pool(name="psc", bufs=3, space="PSUM")
    pz = pool(name="pz", bufs=1, space="PSUM")

    # ---- persistent SBUF tensors ----
    kT = pconst.tile([64, NT, 2, 128], E4, tag="kT")
    vb = pconst.tile([128, NT, 2, 129], E4, tag="vb")     # PV pairs
    qT = pconst.tile([64, N_QC, 2, QC], E4, tag="qT")
    ckT = pconst.tile([64, NCT, 2, 128], E4, tag="ckT")
    cvb = pconst.tile([128, NCT // 2, 2, 129], E4, tag="cvb")
    zcomp = pconst.tile([128, Q // 128, 128], F32, tag="zcomp")  # [q, qt, d]
    accq = pconst.tile([128, Q // 128], F32, tag="accq")

    # Warm the ACT exp table so ACT_TABLE_LOAD is off the critical path.
    # nshift doubles as the per-partition bias AP for the Exp activations.
    nshift = pconst.tile([128, 1], F32, tag="nshift")
    nc.gpsimd.memset(nshift[:], -SHIFT)
    warm2 = psmall.tile([128, 1], F32, tag="warm2")
    nc.scalar.activation(warm2[:], nshift[:], AF.Exp)

    # Warm the PE HAM clock gate with dummy DoubleRow matmuls during the
    # input-DMA lead so the first real matmul runs at full clock.
    wb = psmall.tile([64, 2, 256], E4, tag="wb")
    nc.gpsimd.memset(wb[:], 0.0)
    for _ in range(16):
        wps = psc.tile([128, GT, QC], F32, tag="sp")
        nc.tensor.matmul(wps[:, 0, :], wb[:, :, 0:128], wb[:],
                         start=True, stop=True, perf_mode=DR)

    # ---- input DMAs (pre-swizzled e4m3) ----
    KCH = 8
    kt_per = NT // KCH

    def kchunk(c):
        return (kT[:, c * kt_per:(c + 1) * kt_per],
                kTh[:, c * kt_per * 256:(c + 1) * kt_per * 256]
                .rearrange("p (t i n) -> p t i n", t=kt_per, i=2))

    def vchunk(c):
        vt_per = kt_per // 2
        return (vb[:, c * vt_per:(c + 1) * vt_per],
                vbh[:, c * vt_per * 258:(c + 1) * vt_per * 258]
                .rearrange("p (t i n) -> p t i n", t=vt_per, i=2))

    nc.sync.dma_start(out=ckT[:],
                      in_=ckTh[:].rearrange("p (t i n) -> p t i n", t=NCT, i=2))
    nc.sync.dma_start(out=qT[:, 0:1],
                      in_=qTh[:, 0:2 * QC].rearrange("p (c i n) -> p c i n", c=1, i=2))
    nc.sync.dma_start(out=cvb[:],
                      in_=cvbh[:].rearrange("p (t i n) -> p t i n", t=NCT // 2, i=2))
    nc.sync.dma_start(out=qT[:, 1:N_QC],
                      in_=qTh[:, 2 * QC:].rearrange("p (c i n) -> p c i n",
                                                    c=N_QC - 1, i=2))
    stream = [kchunk(0), kchunk(1), vchunk(0), kchunk(2), vchunk(1),
              kchunk(3), vchunk(2), kchunk(4), vchunk(3), kchunk(5),
              vchunk(4), kchunk(6), vchunk(5), kchunk(7), vchunk(6),
              vchunk(7)]
    for o, i in stream:
        nc.sync.dma_start(out=o, in_=i)

    # ---- attention + softmax-PV for one q-chunk of 256 ----
    def attend(keysT, vals, n_tiles, qc, on_dve, hooks=None):
        """Returns (za, zb) PSUM tiles [128, 129] = [z' | S] per q-half."""
        za = pz.tile([128, 129], F32, tag="za")
        zb = pz.tile([128, 129], F32, tag="zb")
        qs = qT[:, qc]                                   # [64, 2, 256]
        n_pairs = n_tiles // 2

        def emit_pv(ex, t0, gn):
            for pj in range(gn // 2):
                tp = t0 // 2 + pj
                st = dict(start=(tp == 0), stop=(tp == n_pairs - 1))
                for c0, zp in ((0, za), (128, zb)):
                    nc.tensor.matmul(zp[:], ex[:, 2 * pj:2 * pj + 2, c0:c0 + 128],
                                     vals[:, tp], perf_mode=DR, **st)

        n_groups = n_tiles // GT
        pending = []
        for gi in range(n_groups):
            t0 = gi * GT
            sp = psc.tile([128, GT, QC], F32, tag="sp")
            for j in range(GT):
                nc.tensor.matmul(sp[:, j, :], keysT[:, t0 + j], qs,
                                 start=True, stop=True, perf_mode=DR)
            if len(pending) >= PV_LAG:
                emit_pv(*pending.pop(0))
            ex = pex.tile([128, GT, QC], E5, tag="ex")
            if on_dve(qc, gi):
                nc.vector.tensor_scalar(ex[:].bitcast(I8), sp[:], A5, B5,
                                        op0=ALU.mult, op1=ALU.add)
            else:
                nc.scalar.activation(ex[:], sp[:], AF.Exp,
                                     scale=SCALE, bias=nshift[:])
            if hooks and gi in hooks:
                hooks[gi]()
            pending.append((ex, t0, GT))
        for p in pending:
            emit_pv(*p)
        return za, zb

    def drain_hook(zp_pair, zs_pair):
        def run():
            for zp, zs in zip(zp_pair, zs_pair):
                nc.scalar.copy(zs[:], zp[:])
        return run

    def zcomp_normalize(czs, qt):
        def run():
            inv = psmall.tile([128, 1], F32, tag="cinv")
            nc.vector.reciprocal(inv[:], czs[:, 128:129])
            nc.vector.tensor_scalar_mul(zcomp[:, qt, :], czs[:, 0:128], inv[:])
        return run

    def mse_dve(zs, qt):
        inv = psmall.tile([128, 1], F32, tag="inv")
        nc.vector.reciprocal(inv[:], zs[:, 128:129])
        d = psmall.tile([128, 128], F32, tag="d")
        nc.vector.scalar_tensor_tensor(d[:], zs[:, 0:128], inv[:],
                                       zcomp[:, qt, :],
                                       op0=ALU.mult, op1=ALU.subtract)
        d2 = psmall.tile([128, 128], F32, tag="d2")
        nc.vector.scalar_tensor_tensor(d2[:], d[:], 1.0, d[:],
                                       op0=ALU.mult, op1=ALU.mult,
                                       accum_out=accq[:, qt:qt + 1])

    def mse_hook(zs, qt):
        def run():
            mse_dve(zs, qt)
        return run

    # Interleaved phases per q-chunk (see module docstring).
    prev_mse = []
    prev_tz = None
    for qc in range(N_QC):
        chooks = {}
        if prev_tz is not None:
            zs0 = psmall.tile([128, 129], F32, tag="zs")
            zs1 = psmall.tile([128, 129], F32, tag="zs")
            zs = [zs0, zs1]
            chooks[1] = drain_hook(prev_tz, zs)
            prev_mse = [mse_hook(zs[0], (qc - 1) * 2), mse_hook(zs[1], (qc - 1) * 2 + 1)]
        cza, czb = attend(ckT, cvb, NCT, qc, comp_on_dve, chooks)

        czs0 = psmall.tile([128, 129], F32, tag="czs")
        czs1 = psmall.tile([128, 129], F32, tag="czs")
        czs = [czs0, czs1]
        hooks = {1: drain_hook((cza, czb), czs),
                 6: zcomp_normalize(czs[0], qc * 2),
                 7: zcomp_normalize(czs[1], qc * 2 + 1)}
        for i, fn in enumerate(prev_mse):
            hooks[9 + 3 * i] = fn
        za, zb = attend(kT, vb, NT, qc, teacher_on_dve, hooks)
        prev_tz = (za, zb)

    # last chunk: nothing reuses the PSUM banks; run MSE off PSUM on DVE.
    zsl0 = psmall.tile([128, 129], F32, tag="zs")
    zsl1 = psmall.tile([128, 129], F32, tag="zs")
    nc.scalar.copy(zsl0[:], prev_tz[0][:])
    nc.scalar.copy(zsl1[:], prev_tz[1][:])
    mse_dve(zsl0, (N_QC - 1) * 2)
    mse_dve(zsl1, (N_QC - 1) * 2 + 1)

    nc.sync.dma_start(out=out_dram[:], in_=accq[:])

    for p in reversed(ctxs):
        p.__exit__(None, None, None)


_NC_CACHE = None


def build_nc():
    global _NC_CACHE
    if _NC_CACHE is not None:
        return _NC_CACHE
    nc = bacc.Bacc()
    qTh = nc.declare_dram_parameter("qT", [64, N_QC * 2 * QC], E4, isOutput=False)
    kTh = nc.declare_dram_parameter("kT", [64, NT * 2 * 128], E4, isOutput=False)
    vbh = nc.declare_dram_parameter("vb", [128, NT * 129], E4, isOutput=False)
    ckTh = nc.declare_dram_parameter("ckT", [64, NCT * 2 * 128], E4, isOutput=False)
    cvbh = nc.declare_dram_parameter("cvb", [128, NCT * 129], E4, isOutput=False)
    out = nc.declare_dram_parameter("loss_sums", [128, Q // 128], F32, isOutput=True)
    with tile.TileContext(nc) as tc:
        _emit(nc, tc, qTh, kTh, vbh, ckTh, cvbh, out)
    nc.compile()
    _NC_CACHE = nc
    return nc


NP_E4 = ml_dtypes.float8_e4m3


def _prep_head(qh, kh, vh, ckh, cvh):
    """Host-side shard prep: swizzle/cast one head's operands to fp8."""
    def split_d(x, tiles):       # [n, 128d] -> [64p, tiles, 2, 128n]
        t = x.shape[0] // 128
        xt = x.T.reshape(2, 64, t, 128).transpose(1, 2, 0, 3)
        return np.ascontiguousarray(xt).astype(NP_E4).reshape(64, -1)

    def swizzle_v(v):            # [n, d] -> [128p, t, d+1] with ones col
        t = v.shape[0] // 128
        vs = v.reshape(t, 128, D).transpose(1, 0, 2)
        out = np.empty((128, t, D + 1), dtype=NP_E4)
        out[:, :, 0:D] = vs.astype(NP_E4)
        out[:, :, D] = np.asarray(1.0, dtype=NP_E4)
        return out.reshape(128, t * (D + 1))

    def split_q(q):              # [1024, 128d] -> [64p, qc, 2, 256]
        qt = q.T.reshape(2, 64, N_QC, QC).transpose(1, 2, 0, 3)
        return np.ascontiguousarray(qt).astype(NP_E4).reshape(64, -1)

    return {
        "qT": split_q(qh),
        "kT": split_d(kh, NT),
        "vb": swizzle_v(vh),
        "ckT": split_d(ckh, NCT),
        "cvb": swizzle_v(cvh),
    }


def make_in_maps(queries, keys, values, c_keys, c_values):
    in_maps = []
    for h in range(N_CORES):
        in_maps.append(_prep_head(
            np.asarray(queries[0, h], dtype=np.float32),
            np.asarray(keys[0, h], dtype=np.float32),
            np.asarray(values[0, h], dtype=np.float32),
            np.asarray(c_keys[0, h], dtype=np.float32),
            np.asarray(c_values[0, h], dtype=np.float32),
        ))
    return in_maps


def run_cores(in_maps, trace=False, **kw):
    nc = build_nc()
    return run_bass_kernel_spmd(nc, in_maps, list(range(N_CORES)),
                                trace=trace, **kw)


def kernel(queries, keys, values, c_keys, c_values):
    res = run_cores(make_in_maps(queries, keys, values, c_keys, c_values))
    total = sum(float(r["loss_sums"].astype(np.float64).sum())
                for r in res.results)
    loss = total / float(B * H * Q * D)
    return np.asarray(loss, dtype=np.float32)


# revision 11
# speedup vs baseline: 1.6433x; 1.6433x over previous
"""Distillation-trainer loss kernel for Trainium2 (8 NeuronCores).

Computes  loss = mean((attn(q,k,v) - attn(q,ck,cv))**2)  for
q:[1,8,1024,128], k/v:[1,8,8192,128], ck/cv:[1,8,1024,128] fp32.

Sharding: one kv-head per core (h axis, 8 heads / 8 cores). Each core
computes its head's squared-error partial sums; the host adds the 8
partials and divides by the element count (the "all-reduce" of the
scalar loss).

Host-side prep (part of sharding): per head, ship bf16 operands in the
exact SBUF layouts the PE needs — kT/qT/ckT pre-transposed to [d, n],
v/cv pre-swizzled to [128p, t, d] with a ones column appended (the
denominator trick). This removes all on-device transposes/casts and
halves DMA bytes. The input stream is issued on one queue in
just-in-time consumption order (the queue acts as a priority list).

Per-core algorithm (head h), per 256-wide q-chunk:
  - scoresT[n, q] = kT-tile.T @ qT-chunk on PE in bf16 (fp32 PSUM).
    Scores grouped 4 n-tiles (2 PSUM banks) x 3 buffers so TWO exp
    engines run concurrently on different groups:
      ACT:  expT = Exp(scoresT * 1/sqrt(d)) -> bf16 (even groups).
      DVE:  Schraudolph in bf16 (odd groups): i16 = rint(s*A16 + B16)
            written through a bitcast into the bf16 tile; the int16 bit
            pattern IS the bf16 exp approximation (~2% multiplicative
            noise, zero-mean through softmax; loss rel-err ~3e-4).
  - PV emission lags the QK groups by 2 so exp latency (~1.2-1.5us) is
    hidden behind ~1.8us of PE work: stationary = expT chunk
    [128n, 128q], moving = v' [128n, 129]; PSUM accumulates z' | S.
  - ACT copies z'|S PSUM->SBUF right after the PV flush (frees the
    accumulation banks for the next attend with no DVE involvement);
    the DVE normalize/MSE math on those copies is deferred and
    interleaved into the NEXT attend's group loop, keeping the qc
    boundary free of serialized vector work:
      zcomp[qt] = z'c * 1/Sc   (compressed, via reciprocal + mul)
      acc[qt]  += sum((z'*invS - zcomp[qt])^2)  (two fused
                  scalar_tensor_tensor ops, accum_out row sums)
  - Compressed (NC=1024) and teacher (N=8192) attends interleave per
    q-chunk so the kT/vb DMA stream hides behind early compute.
"""

import numpy as np

import concourse.bass as bass
import concourse.mybir as mybir
import concourse.tile as tile
from concourse import bacc
from concourse.bass_utils import run_bass_kernel_spmd

F32 = mybir.dt.float32
BF16 = mybir.dt.bfloat16
I16 = mybir.dt.int16
AF = mybir.ActivationFunctionType
ALU = mybir.AluOpType

B, H, Q, N, NC, D = 1, 8, 1024, 8192, 1024, 128
N_CORES = 8
SCALE = 1.0 / float(np.sqrt(D))

QC = 256                   # q chunk width for the scores moving operand
N_QC = Q // QC             # 4
GT = 4                     # n-tiles per PSUM scores group (2 banks)
NT = N // 128              # 64 teacher n-tiles
NCT = NC // 128            # 8 compressed n-tiles
PV_LAG = 3                 # == psc bufs: PV(g-3) and QK(g) then wait on the
                           # SAME (sem, value) -> the scheduler dedupes waits

# Schraudolph-to-bf16 constants: exp(s*SCALE) ~= bf16_bits(rint(s*A16+B16)).
# HW DVE converts fp32->int16 with round-to-nearest (measured).
LN2 = float(np.log(2.0))
A16 = float(128.0 / LN2 * SCALE)
B16 = float(127 * 128 - 8)          # b_adj=8 minimizes softmax-weight bias


def teacher_on_dve(qc, gi):
    return gi % 2 == 1


def comp_on_dve(qc, gi):
    return gi == 1


def _emit(nc: bass.Bass, tc: tile.TileContext, qTh, kTh, vbh, ckTh, cvbh, out_dram):
    ctxs = []

    def pool(**kw):
        p = tc.tile_pool(**kw)
        ctxs.append(p)
        return p.__enter__()

    pconst = pool(name="pconst", bufs=1)
    pex = pool(name="pex", bufs=4)
    psmall = pool(name="psmall", bufs=8)
    psc = pool(name="psc", bufs=3, space="PSUM")
    pz = pool(name="pz", bufs=1, space="PSUM")

    # ---- persistent SBUF tensors ----
    kT = pconst.tile([128, NT, 128], BF16, tag="kT")        # [d, t, n]
    vb = pconst.tile([128, NT, 129], BF16, tag="vb")        # [p, t, d+1]
    qT = pconst.tile([128, Q], BF16, tag="qT")              # [d, q]
    ckT = pconst.tile([128, NCT, 128], BF16, tag="ckT")
    cvb = pconst.tile([128, NCT, 129], BF16, tag="cvb")
    zcomp = pconst.tile([128, Q // 128, 128], F32, tag="zcomp")  # [q, qt, d]
    accq = pconst.tile([128, Q // 128], F32, tag="accq")

    # Warm the ACT exp table immediately so the ~2.7us ACT_TABLE_LOAD is
    # off the first real exp's critical path.
    warm = psmall.tile([128, 1], F32, tag="warm")
    nc.gpsimd.memset(warm[:], 0.0)
    warm2 = psmall.tile([128, 1], F32, tag="warm2")
    nc.scalar.activation(warm2[:], warm[:], AF.Exp)

    # Warm the PE HAM clock gate during the input-DMA lead: ~3us of dummy
    # matmuls trips the activity monitor to K=8/8 (2.4 GHz) before the
    # first real matmul instead of ~8us into the compressed phase.
    wb = psmall.tile([128, 512], BF16, tag="wb")
    nc.gpsimd.memset(wb[:], 0.0)
    wps = psc.tile([128, GT, QC], F32, tag="sp")
    for _ in range(8):
        nc.tensor.matmul(wps[0:64, 0:2, :], wb[:, 0:64], wb[:],
                         start=True, stop=True)

    # ---- input DMAs (pre-transposed/swizzled bf16) ----
    # The 5.1MB stream takes ~15us of wire time; one queue, emitted in
    # just-in-time consumption order, acts as a priority schedule. The
    # DMA_DIRECT2D "durations" are descriptor-issue only; actual packets
    # trickle until ~22us, so order is everything here.
    KCH = 8
    kt_per = NT // KCH

    def kchunk(c):
        return (kT[:, c * kt_per:(c + 1) * kt_per, :],
                kTh[:, c * kt_per * 128:(c + 1) * kt_per * 128]
                .rearrange("p (t n) -> p t n", t=kt_per))

    def vchunk(c):
        return (vb[:, c * kt_per:(c + 1) * kt_per, :],
                vbh[:, c * kt_per * 129:(c + 1) * kt_per * 129]
                .rearrange("p (t n) -> p t n", t=kt_per))

    nc.sync.dma_start(out=ckT[:],
                      in_=ckTh[:, :].rearrange("p (t n) -> p t n", t=NCT))
    nc.sync.dma_start(out=qT[:, 0:QC], in_=qTh[:, 0:QC])
    nc.sync.dma_start(out=cvb[:],
                      in_=cvbh[:, :].rearrange("p (t n) -> p t n", t=NCT))
    nc.sync.dma_start(out=qT[:, QC:Q], in_=qTh[:, QC:Q])
    stream = [kchunk(0), kchunk(1), vchunk(0), kchunk(2), vchunk(1),
              kchunk(3), vchunk(2), kchunk(4), vchunk(3), kchunk(5),
              vchunk(4), kchunk(6), vchunk(5), kchunk(7), vchunk(6),
              vchunk(7)]
    for o, i in stream:
        nc.sync.dma_start(out=o, in_=i)

    # ---- attention + softmax-PV for one q-chunk of 256 ----
    def attend(keysT, vals, n_tiles, qc, on_dve, hooks=None):
        """Returns (za, zb) PSUM tiles [128, 129] = [z' | S] per q-half."""
        za = pz.tile([128, 129], F32, tag="za")
        zb = pz.tile([128, 129], F32, tag="zb")
        qs = qT[:, qc * QC:(qc + 1) * QC]

        def emit_pv(ex, t0, gn):
            for j in range(gn):
                t = t0 + j
                st = dict(start=(t == 0), stop=(t == n_tiles - 1))
                for c0, zp in ((0, za), (128, zb)):
                    nc.tensor.matmul(zp[:], ex[:, j, c0:c0 + 128],
                                     vals[:, t, :], **st)

        n_groups = n_tiles // GT
        pending = []
        for gi in range(n_groups):
            t0 = gi * GT
            sp = psc.tile([128, GT, QC], F32, tag="sp")
            for j in range(GT):
                nc.tensor.matmul(sp[:, j, :], keysT[:, t0 + j, :], qs,
                                 start=True, stop=True)
            if len(pending) >= PV_LAG:
                emit_pv(*pending.pop(0))
            ex = pex.tile([128, GT, QC], BF16, tag="ex")
            if on_dve(qc, gi):
                nc.vector.tensor_scalar(ex[:].bitcast(I16), sp[:], A16, B16,
                                        op0=ALU.mult, op1=ALU.add)
            else:
                nc.scalar.activation(ex[:], sp[:], AF.Exp, scale=SCALE)
            if hooks and gi in hooks:
                hooks[gi]()
            pending.append((ex, t0, GT))
        for p in pending:
            emit_pv(*p)
        return za, zb

    def act_drain(zp_pair):
        """ACT copies z'|S PSUM->SBUF, freeing the accumulation banks."""
        out = []
        for zp in zp_pair:
            zs = psmall.tile([128, 129], F32, tag="zs")
            nc.scalar.copy(zs[:], zp[:])
            out.append(zs)
        return out

    def zcomp_normalize(czs, qt):
        def run():
            inv = psmall.tile([128, 1], F32, tag="cinv")
            nc.vector.reciprocal(inv[:], czs[:, 128:129])
            nc.vector.tensor_scalar_mul(zcomp[:, qt, :], czs[:, 0:128], inv[:])
        return run

    def mse(zs, qt):
        def run():
            inv = psmall.tile([128, 1], F32, tag="inv")
            nc.vector.reciprocal(inv[:], zs[:, 128:129])
            d = psmall.tile([128, 128], F32, tag="d")
            nc.vector.scalar_tensor_tensor(d[:], zs[:, 0:128], inv[:],
                                           zcomp[:, qt, :],
                                           op0=ALU.mult, op1=ALU.subtract)
            d2 = psmall.tile([128, 128], F32, tag="d2")
            nc.vector.scalar_tensor_tensor(d2[:], d[:], 1.0, d[:],
                                           op0=ALU.mult, op1=ALU.mult,
                                           accum_out=accq[:, qt:qt + 1])
        return run

    def drain_hook(zp_pair, zs_pair):
        def run():
            for zp, zs in zip(zp_pair, zs_pair):
                nc.scalar.copy(zs[:], zp[:])
        return run

    # Phase 1: ALL compressed attends run first, back to back, while the
    # big kT/vb stream is still in flight -- the 7.7us of comp matmuls
    # cover the DMA wire time so the teacher phase never races the DMAs.
    # Drains of comp z hook into the NEXT comp attend; normalize runs on
    # the (idle-in-this-phase) DVE right after.
    prev_cz = None
    for qc in range(N_QC):
        chooks = {}
        post = []
        if prev_cz is not None:
            czs0 = psmall.tile([128, 129], F32, tag="czs")
            czs1 = psmall.tile([128, 129], F32, tag="czs")
            chooks[1] = drain_hook(prev_cz, (czs0, czs1))
            post = [zcomp_normalize(czs0, (qc - 1) * 2),
                    zcomp_normalize(czs1, (qc - 1) * 2 + 1)]
        prev_cz = attend(ckT, cvb, NCT, qc, comp_on_dve, chooks)
        for fn in post:
            fn()

    # Phase 2: teacher attends per q-chunk; the previous chunk's z'|S
    # drain + MSE hook into the group loop as before.
    prev_tz = None
    for qc in range(N_QC):
        if qc == 0:
            czs0 = psmall.tile([128, 129], F32, tag="czs")
            czs1 = psmall.tile([128, 129], F32, tag="czs")
            hooks = {1: drain_hook(prev_cz, (czs0, czs1)),
                     6: zcomp_normalize(czs0, (N_QC - 1) * 2),
                     7: zcomp_normalize(czs1, (N_QC - 1) * 2 + 1)}
        else:
            zs0 = psmall.tile([128, 129], F32, tag="zs")
            zs1 = psmall.tile([128, 129], F32, tag="zs")
            hooks = {1: drain_hook(prev_tz, (zs0, zs1)),
                     9: mse(zs0, (qc - 1) * 2),
                     12: mse(zs1, (qc - 1) * 2 + 1)}
        za, zb = attend(kT, vb, NT, qc, teacher_on_dve, hooks)
        prev_tz = (za, zb)

    # last chunk: nothing reuses the PSUM banks, so run the MSE straight
    # off PSUM and skip the copy latency.
    mse(prev_tz[0], (N_QC - 1) * 2)()
    mse(prev_tz[1], (N_QC - 1) * 2 + 1)()

    nc.sync.dma_start(out=out_dram[:], in_=accq[:])

    for p in reversed(ctxs):
        p.__exit__(None, None, None)


_NC_CACHE = None


def build_nc():
    global _NC_CACHE
    if _NC_CACHE is not None:
        return _NC_CACHE
    nc = bacc.Bacc()
    qTh = nc.declare_dram_parameter("qT", [128, Q], BF16, isOutput=False)
    kTh = nc.declare_dram_parameter("kT", [128, N], BF16, isOutput=False)
    vbh = nc.declare_dram_parameter("vb", [128, NT * 129], BF16, isOutput=False)
    ckTh = nc.declare_dram_parameter("ckT", [128, NC], BF16, isOutput=False)
    cvbh = nc.declare_dram_parameter("cvb", [128, NCT * 129], BF16, isOutput=False)
    out = nc.declare_dram_parameter("loss_sums", [128, Q // 128], F32, isOutput=True)
    with tile.TileContext(nc) as tc:
        _emit(nc, tc, qTh, kTh, vbh, ckTh, cvbh, out)
    nc.compile()
    _NC_CACHE = nc
    return nc


NPBF16 = mybir.dt.np(BF16)


def _prep_head(qh, kh, vh, ckh, cvh):
    """Host-side shard prep: transpose/swizzle/cast one head's operands."""
    def swizzle_v(v):              # [n, d] -> [128p, t, d+1] with ones col
        t = v.shape[0] // 128
        vs = v.reshape(t, 128, D).transpose(1, 0, 2)
        out = np.empty((128, t, D + 1), dtype=NPBF16)
        out[:, :, 0:D] = vs.astype(NPBF16)
        out[:, :, D] = np.asarray(1.0, dtype=NPBF16)
        return out.reshape(128, t * (D + 1))

    return {
        "qT": np.ascontiguousarray(qh.T).astype(NPBF16),
        "kT": np.ascontiguousarray(kh.T).astype(NPBF16),
        "vb": swizzle_v(vh),
        "ckT": np.ascontiguousarray(ckh.T).astype(NPBF16),
        "cvb": swizzle_v(cvh),
    }


def make_in_maps(queries, keys, values, c_keys, c_values):
    in_maps = []
    for h in range(N_CORES):
        in_maps.append(_prep_head(
            np.asarray(queries[0, h], dtype=np.float32),
            np.asarray(keys[0, h], dtype=np.float32),
            np.asarray(values[0, h], dtype=np.float32),
            np.asarray(c_keys[0, h], dtype=np.float32),
            np.asarray(c_values[0, h], dtype=np.float32),
        ))
    return in_maps


def run_cores(in_maps, trace=False, **kw):
    nc = build_nc()
    return run_bass_kernel_spmd(nc, in_maps, list(range(N_CORES)),
                                trace=trace, **kw)


def kernel(queries, keys, values, c_keys, c_values):
    res = run_cores(make_in_maps(queries, keys, values, c_keys, c_values))
    total = sum(float(r["loss_sums"].astype(np.float64).sum())
                for r in res.results)
    loss = total / float(B * H * Q * D)
    return np.asarray(loss, dtype=np.float32)



# revision 14
# speedup vs baseline: 1.6734x; 1.0184x over previous
"""Distillation-trainer loss kernel for Trainium2 (8 NeuronCores).

Computes  loss = mean((attn(q,k,v) - attn(q,ck,cv))**2)  for
q:[1,8,1024,128], k/v:[1,8,8192,128], ck/cv:[1,8,1024,128] fp32.

Sharding: one kv-head per core (h axis, 8 heads / 8 cores). Each core
computes its head's squared-error partial sums; the host adds the 8
partials and divides by the element count (the "all-reduce" of the
scalar loss).

Host-side prep (part of sharding): per head, ship bf16 operands in the
exact SBUF layouts the PE needs — kT/qT/ckT pre-transposed to [d, n],
v/cv pre-swizzled to [128p, t, d] with a ones column appended (the
denominator trick). This removes all on-device transposes/casts and
halves DMA bytes. The input stream is issued on one queue in
just-in-time consumption order (the queue acts as a priority list).

Per-core algorithm (head h), per 256-wide q-chunk:
  - scoresT[n, q] = kT-tile.T @ qT-chunk on PE in bf16 (fp32 PSUM).
    Scores grouped 4 n-tiles (2 PSUM banks) x 3 buffers so TWO exp
    engines run concurrently on different groups:
      ACT:  expT = Exp(scoresT * 1/sqrt(d)) -> bf16 (even groups).
      DVE:  Schraudolph in bf16 (odd groups): i16 = rint(s*A16 + B16)
            written through a bitcast into the bf16 tile; the int16 bit
            pattern IS the bf16 exp approximation (~2% multiplicative
            noise, zero-mean through softmax; loss rel-err ~3e-4).
  - PV emission lags the QK groups by 2 so exp latency (~1.2-1.5us) is
    hidden behind ~1.8us of PE work: stationary = expT chunk
    [128n, 128q], moving = v' [128n, 129]; PSUM accumulates z' | S.
  - ACT copies z'|S PSUM->SBUF right after the PV flush (frees the
    accumulation banks for the next attend with no DVE involvement);
    the DVE normalize/MSE math on those copies is deferred and
    interleaved into the NEXT attend's group loop, keeping the qc
    boundary free of serialized vector work:
      zcomp[qt] = z'c * 1/Sc   (compressed, via reciprocal + mul)
      acc[qt]  += sum((z'*invS - zcomp[qt])^2)  (two fused
                  scalar_tensor_tensor ops, accum_out row sums)
  - Compressed (NC=1024) and teacher (N=8192) attends interleave per
    q-chunk so the kT/vb DMA stream hides behind early compute.
"""

import numpy as np

import concourse.bass as bass
import concourse.mybir as mybir
import concourse.tile as tile
from concourse import bacc
from concourse.bass_utils import run_bass_kernel_spmd

F32 = mybir.dt.float32
BF16 = mybir.dt.bfloat16
I16 = mybir.dt.int16
AF = mybir.ActivationFunctionType
ALU = mybir.AluOpType

B, H, Q, N, NC, D = 1, 8, 1024, 8192, 1024, 128
N_CORES = 8
SCALE = 1.0 / float(np.sqrt(D))

QC = 256                   # q chunk width for the scores moving operand
N_QC = Q // QC             # 4
GT = 4                     # n-tiles per PSUM scores group (2 banks)
NT = N // 128              # 64 teacher n-tiles
NCT = NC // 128            # 8 compressed n-tiles
PV_LAG = 3                 # == psc bufs: PV(g-3) and QK(g) then wait on the
                           # SAME (sem, value) -> the scheduler dedupes waits

# Schraudolph-to-bf16 constants: exp(s*SCALE) ~= bf16_bits(rint(s*A16+B16)).
# HW DVE converts fp32->int16 with round-to-nearest (measured).
LN2 = float(np.log(2.0))
A16 = float(128.0 / LN2 * SCALE)
B16 = float(127 * 128 - 8)          # b_adj=8 minimizes softmax-weight bias


def teacher_on_dve(qc, gi):
    return gi % 2 == 1


def comp_on_dve(qc, gi):
    return gi == 1


def _emit(nc: bass.Bass, tc: tile.TileContext, qTh, kTh, vbh, ckTh, cvbh, out_dram):
    ctxs = []

    def pool(**kw):
        p = tc.tile_pool(**kw)
        ctxs.append(p)
        return p.__enter__()

    pconst = pool(name="pconst", bufs=1)
    pex = pool(name="pex", bufs=3)
    psmall = pool(name="psmall", bufs=6)
    psc = pool(name="psc", bufs=3, space="PSUM")
    pz = pool(name="pz", bufs=1, space="PSUM")

    # ---- persistent SBUF tensors ----
    kT = pconst.tile([128, NT, 128], BF16, tag="kT")        # [d, t, n]
    vb = pconst.tile([128, NT, 129], BF16, tag="vb")        # [p, t, d+1]
    qT = pconst.tile([128, Q], BF16, tag="qT")              # [d, q]
    ckT = pconst.tile([128, NCT, 128], BF16, tag="ckT")
    cvb = pconst.tile([128, NCT, 129], BF16, tag="cvb")
    zcomp = pconst.tile([128, Q // 128, 128], F32, tag="zcomp")  # [q, qt, d]
    accq = pconst.tile([128, Q // 128], F32, tag="accq")

    # Warm the ACT exp table immediately so the ~2.7us ACT_TABLE_LOAD is
    # off the first real exp's critical path.
    warm = psmall.tile([128, 1], F32, tag="warm")
    nc.gpsimd.memset(warm[:], 0.0)
    warm2 = psmall.tile([128, 1], F32, tag="warm2")
    nc.scalar.activation(warm2[:], warm[:], AF.Exp)

    # Warm the PE HAM clock gate during the input-DMA lead: ~3us of dummy
    # matmuls trips the activity monitor to K=8/8 (2.4 GHz) before the
    # first real matmul instead of ~8us into the compressed phase.
    wb = psmall.tile([128, 512], BF16, tag="wb")
    nc.gpsimd.memset(wb[:], 0.0)
    wps = psc.tile([128, GT, QC], F32, tag="sp")
    for _ in range(8):
        nc.tensor.matmul(wps[0:64, 0:2, :], wb[:, 0:64], wb[:],
                         start=True, stop=True)

    # ---- input DMAs (pre-transposed/swizzled bf16) ----
    # The 5.1MB stream takes ~15us of wire time; one queue, emitted in
    # just-in-time consumption order, acts as a priority schedule. The
    # DMA_DIRECT2D "durations" are descriptor-issue only; actual packets
    # trickle until ~22us, so order is everything here.
    KCH = 8
    kt_per = NT // KCH

    def kchunk(c):
        return (kT[:, c * kt_per:(c + 1) * kt_per, :],
                kTh[:, c * kt_per * 128:(c + 1) * kt_per * 128]
                .rearrange("p (t n) -> p t n", t=kt_per))

    def vchunk(c):
        return (vb[:, c * kt_per:(c + 1) * kt_per, :],
                vbh[:, c * kt_per * 129:(c + 1) * kt_per * 129]
                .rearrange("p (t n) -> p t n", t=kt_per))

    nc.sync.dma_start(out=ckT[:],
                      in_=ckTh[:, :].rearrange("p (t n) -> p t n", t=NCT))
    nc.sync.dma_start(out=qT[:, 0:QC], in_=qTh[:, 0:QC])
    nc.sync.dma_start(out=cvb[:],
                      in_=cvbh[:, :].rearrange("p (t n) -> p t n", t=NCT))
    nc.sync.dma_start(out=qT[:, QC:Q], in_=qTh[:, QC:Q])
    stream = [kchunk(0), kchunk(1), vchunk(0), kchunk(2), vchunk(1),
              kchunk(3), vchunk(2), kchunk(4), vchunk(3), kchunk(5),
              vchunk(4), kchunk(6), vchunk(5), kchunk(7), vchunk(6),
              vchunk(7)]
    for o, i in stream:
        nc.sync.dma_start(out=o, in_=i)

    # ---- attention + softmax-PV for one q-chunk of 256 ----
    def attend(keysT, vals, n_tiles, qc, on_dve, hooks=None):
        """Returns (za, zb) PSUM tiles [128, 129] = [z' | S] per q-half."""
        za = pz.tile([128, 129], F32, tag="za")
        zb = pz.tile([128, 129], F32, tag="zb")
        qs = qT[:, qc * QC:(qc + 1) * QC]

        def emit_pv(ex, t0, gn):
            for j in range(gn):
                t = t0 + j
                st = dict(start=(t == 0), stop=(t == n_tiles - 1))
                for c0, zp in ((0, za), (128, zb)):
                    nc.tensor.matmul(zp[:], ex[:, j, c0:c0 + 128],
                                     vals[:, t, :], **st)

        n_groups = n_tiles // GT
        pending = []
        for gi in range(n_groups):
            t0 = gi * GT
            sp = psc.tile([128, GT, QC], F32, tag="sp")
            for j in range(GT):
                nc.tensor.matmul(sp[:, j, :], keysT[:, t0 + j, :], qs,
                                 start=True, stop=True)
            if len(pending) >= PV_LAG:
                emit_pv(*pending.pop(0))
            ex = pex.tile([128, GT, QC], BF16, tag="ex")
            if on_dve(qc, gi):
                nc.vector.tensor_scalar(ex[:].bitcast(I16), sp[:], A16, B16,
                                        op0=ALU.mult, op1=ALU.add)
            else:
                nc.scalar.activation(ex[:], sp[:], AF.Exp, scale=SCALE)
            if hooks and gi in hooks:
                hooks[gi]()
            pending.append((ex, t0, GT))
        for p in pending:
            emit_pv(*p)
        return za, zb

    def act_drain(zp_pair):
        """ACT copies z'|S PSUM->SBUF, freeing the accumulation banks."""
        out = []
        for zp in zp_pair:
            zs = psmall.tile([128, 129], F32, tag="zs")
            nc.scalar.copy(zs[:], zp[:])
            out.append(zs)
        return out

    def zcomp_normalize(czs, qt):
        def run():
            inv = psmall.tile([128, 1], F32, tag="cinv")
            nc.vector.reciprocal(inv[:], czs[:, 128:129])
            nc.vector.tensor_scalar_mul(zcomp[:, qt, :], czs[:, 0:128], inv[:])
        return run

    def mse(zs, qt):
        def run():
            inv = psmall.tile([128, 1], F32, tag="inv")
            nc.vector.reciprocal(inv[:], zs[:, 128:129])
            d = psmall.tile([128, 128], F32, tag="d")
            nc.vector.scalar_tensor_tensor(d[:], zs[:, 0:128], inv[:],
                                           zcomp[:, qt, :],
                                           op0=ALU.mult, op1=ALU.subtract)
            d2 = psmall.tile([128, 128], F32, tag="d2")
            nc.vector.scalar_tensor_tensor(d2[:], d[:], 1.0, d[:],
                                           op0=ALU.mult, op1=ALU.mult,
                                           accum_out=accq[:, qt:qt + 1])
        return run

    def drain_hook(zp_pair, zs_pair):
        def run():
            for zp, zs in zip(zp_pair, zs_pair):
                nc.scalar.copy(zs[:], zp[:])
        return run

    # Phase 1: ALL compressed attends run first, back to back, while the
    # big kT/vb stream is still in flight -- the 7.7us of comp matmuls
    # cover the DMA wire time so the teacher phase never races the DMAs.
    # Drains of comp z hook into the NEXT comp attend; normalize runs on
    # the (idle-in-this-phase) DVE right after.
    # The z'|S drain is emitted right AFTER each attend: at that point the
    # ACT queue head waits only on the attend's own PV flush, during which
    # ACT is idle anyway, and the drain completes well before the next
    # attend's first PV needs the banks back (no boundary stall).
    prev_cz = None
    for qc in range(N_QC):
        post = []
        prev_cz = attend(ckT, cvb, NCT, qc, comp_on_dve, None)
        czs0 = psmall.tile([128, 129], F32, tag="czs")
        czs1 = psmall.tile([128, 129], F32, tag="czs")
        drain_hook(prev_cz, (czs0, czs1))()
        zcomp_normalize(czs0, qc * 2)()
        zcomp_normalize(czs1, qc * 2 + 1)()

    # Phase 2: teacher attends per q-chunk; the previous chunk's MSE
    # hooks into the group loop as before.
    prev_zs = None
    prev_tz = None
    for qc in range(N_QC):
        hooks = {}
        if prev_zs is not None:
            hooks = {9: mse(prev_zs[0], (qc - 1) * 2),
                     12: mse(prev_zs[1], (qc - 1) * 2 + 1)}
        za, zb = attend(kT, vb, NT, qc, teacher_on_dve, hooks)
        prev_tz = (za, zb)
        if qc < N_QC - 1:
            zs0 = psmall.tile([128, 129], F32, tag="zs")
            zs1 = psmall.tile([128, 129], F32, tag="zs")
            drain_hook(prev_tz, (zs0, zs1))()
            prev_zs = (zs0, zs1)

    # last chunk: nothing reuses the PSUM banks, so run the MSE straight
    # off PSUM and skip the copy latency.
    mse(prev_tz[0], (N_QC - 1) * 2)()
    mse(prev_tz[1], (N_QC - 1) * 2 + 1)()

    nc.sync.dma_start(out=out_dram[:], in_=accq[:])

    for p in reversed(ctxs):
        p.__exit__(None, None, None)


_NC_CACHE = None


def build_nc():
    global _NC_CACHE
    if _NC_CACHE is not None:
        return _NC_CACHE
    nc = bacc.Bacc()
    qTh = nc.declare_dram_parameter("qT", [128, Q], BF16, isOutput=False)
    kTh = nc.declare_dram_parameter("kT", [128, N], BF16, isOutput=False)
    vbh = nc.declare_dram_parameter("vb", [128, NT * 129], BF16, isOutput=False)
    ckTh = nc.declare_dram_parameter("ckT", [128, NC], BF16, isOutput=False)
    cvbh = nc.declare_dram_parameter("cvb", [128, NCT * 129], BF16, isOutput=False)
    out = nc.declare_dram_parameter("loss_sums", [128, Q // 128], F32, isOutput=True)
    with tile.TileContext(nc) as tc:
        _emit(nc, tc, qTh, kTh, vbh, ckTh, cvbh, out)
    nc.compile()
    _NC_CACHE = nc
    return nc


NPBF16 = mybir.dt.np(BF16)


def _prep_head(qh, kh, vh, ckh, cvh):
    """Host-side shard prep: transpose/swizzle/cast one head's operands."""
    def swizzle_v(v):              # [n, d] -> [128p, t, d+1] with ones col
        t = v.shape[0] // 128
        vs = v.reshape(t, 128, D).transpose(1, 0, 2)
        out = np.empty((128, t, D + 1), dtype=NPBF16)
        out[:, :, 0:D] = vs.astype(NPBF16)
        out[:, :, D] = np.asarray(1.0, dtype=NPBF16)
        return out.reshape(128, t * (D + 1))

    return {
        "qT": np.ascontiguousarray(qh.T).astype(NPBF16),
        "kT": np.ascontiguousarray(kh.T).astype(NPBF16),
        "vb": swizzle_v(vh),
        "ckT": np.ascontiguousarray(ckh.T).astype(NPBF16),
        "cvb": swizzle_v(cvh),
    }


def make_in_maps(queries, keys, values, c_keys, c_values):
    in_maps = []
    for h in range(N_CORES):
        in_maps.append(_prep_head(
            np.asarray(queries[0, h], dtype=np.float32),
            np.asarray(keys[0, h], dtype=np.float32),
            np.asarray(values[0, h], dtype=np.float32),
            np.asarray(c_keys[0, h], dtype=np.float32),
            np.asarray(c_values[0, h], dtype=np.float32),
        ))
    return in_maps


def run_cores(in_maps, trace=False, **kw):
    nc = build_nc()
    return run_bass_kernel_spmd(nc, in_maps, list(range(N_CORES)),
                                trace=trace, **kw)


def kernel(queries, keys, values, c_keys, c_values):
    res = run_cores(make_in_maps(queries, keys, values, c_keys, c_values))
    total = sum(float(r["loss_sums"].astype(np.float64).sum())
                for r in res.results)
    loss = total / float(B * H * Q * D)
    return np.asarray(loss, dtype=np.float32)

